# revision 1
# baseline (speedup 1.0000x reference)
"""Trainium2 Bass kernel for one dense transformer block.

Full (unsharded) IO: x [4, 2048, 1024] -> out [4, 2048, 1024].
Sharding: 8 cores = 4 batches x 2 query-chunk-pair sets. Each core owns one
batch's K/V (2048 rows) and 1024 query rows chosen as causally-balanced
128-row chunk pairs (set A: chunks {4j, 4j+3}, set B: {4j+1, 4j+2}), so every
core runs an identical instruction stream; only data (incl. the causal mask)
differs. No collectives.

On-core dataflow is feature-on-partition ("transposed") throughout:
  LN1 -> hT -> {Q,K}T per head pair -> scores S.T[keys, q] -> exp -> AV with
  an appended ones-column for the softmax denominator -> O.T -> proj ->
  residual -> LN2 -> FFN (streamed W1/W2) -> residual -> transposed DMA out.
Matmuls run in float32r (full PE rate); softmax skips max-subtraction (scores
for this block are O(10); masked lanes get -30000 so exp underflows to 0).
LayerNorm gamma/beta are folded into the weights/biases host-side.
"""

import sys

sys.path.insert(0, "/opt/trn_rl_repo")

import numpy as np

import concourse.bass as bass
import concourse.mybir as mybir
import concourse.tile as tile
from concourse.bass_utils import run_bass_kernel_spmd

f32 = mybir.dt.float32
f32r = mybir.dt.float32r
AL = mybir.AluOpType
AF = mybir.ActivationFunctionType

B, T, C = 4, 2048, 1024
H, D = 16, 64
F = 4 * C
P = 128
TQ = 1024            # query rows per core
NCHUNK = T // P      # 16 chunks of 128 per batch
NEG = -30000.0
LN_EPS = 1e-5


def _split_sync_waits(nc):
    """This container's walrus supports one sync-wait per instruction; Tile
    emits up to ~3. Hoist extras onto NoOps inserted before the owner."""
    ctr = 0
    for fn in nc.m.functions:
        for bb in fn.blocks:
            out, changed = [], False
            for ins in bb.instructions:
                si = ins.sync_info
                waits = list(si.on_wait) if si is not None and si.on_wait else []
                if len(waits) > 1:
                    changed = True
                    for w in waits[:-1]:
                        ctr += 1
                        nop = mybir.InstNoOp(name=f"waitsplit_{ctr}", ins=[], outs=[])
                        nop.engine = ins.engine
                        nop.sync_info = mybir.SyncInfo(on_wait=[w], on_update=[])
                        out.append(nop)
                        nc.register_instruction(nop, overwrite=True)
                    ins.sync_info = mybir.SyncInfo(
                        on_wait=[waits[-1]], on_update=list(si.on_update or [])
                    )
                out.append(ins)
            if changed:
                bb.instructions = out


def _chunk_pairs(s):
    # set A (s=0): (4j, 4j+3); set B (s=1): (4j+1, 4j+2) -- both need
    # key tiles [0, 4j+4) for 256-row local chunk j.
    if s == 0:
        return [(4 * j, 4 * j + 3) for j in range(4)]
    return [(4 * j + 1, 4 * j + 2) for j in range(4)]


def _emit_ln(nc, pools, src_fn, dst, n_rc, ones, eps_sb):
    """LayerNorm over features in transposed layout.

    src_fn(ft, rc) -> [128, 512] f32r AP of input features ft*128.. for row
    chunk rc. dst: [128, 8, n_rc*512] f32r tile receiving (x-mu)*rstd.
    """
    sb, small, ps_s, ps_b = pools
    for rc in range(n_rc):
        # pass 1: stats (x and x^2 tiles are transient)
        psum_m = ps_s.tile([1, 512], f32, tag="ln_m")
        psum_q = ps_s.tile([1, 512], f32, tag="ln_q")
        for ft in range(8):
            xt = src_fn(ft, rc)
            sq = sb.tile([P, 512], f32r, tag="ln_sq")
            nc.vector.tensor_tensor(sq, xt, xt, AL.mult)
            nc.tensor.matmul(psum_m, ones, xt, start=(ft == 0), stop=(ft == 7))
            nc.tensor.matmul(psum_q, ones, sq, start=(ft == 0), stop=(ft == 7))
        mean = small.tile([1, 512], f32, tag="ln_mean")
        nc.vector.tensor_scalar_mul(mean, psum_m, 1.0 / C)
        msq = small.tile([1, 512], f32, tag="ln_msq")
        nc.vector.tensor_scalar_mul(msq, psum_q, 1.0 / C)
        var = small.tile([1, 512], f32, tag="ln_var")
        nc.vector.tensor_tensor(var, mean, mean, AL.mult)
        nc.vector.tensor_tensor(var, msq, var, AL.subtract)
        std = small.tile([1, 512], f32, tag="ln_std")
        nc.scalar.activation(std, var, AF.Sqrt, bias=eps_sb[:, :], scale=1.0)
        a = small.tile([1, 512], f32r, tag="ln_a")
        with nc.allow_low_precision(reason="f32r has f32 bits"):
            nc.vector.reciprocal(a, std)
        bneg = small.tile([1, 512], f32r, tag="ln_b")
        nc.vector.tensor_tensor(bneg, mean, a, AL.mult)
        psum_abc = ps_b.tile([P, 512], f32, tag="ln_abc")
        nc.tensor.matmul(psum_abc, ones[0:1, 0:1].broadcast_to((1, P)), a,
                         start=True, stop=True)
        psum_bbc = ps_b.tile([P, 512], f32, tag="ln_bbc")
        nc.tensor.matmul(psum_bbc, ones[0:1, 0:1].broadcast_to((1, P)), bneg,
                         start=True, stop=True)
        # pass 2: normalize (re-fetch source)
        for ft in range(8):
            xt = src_fn(ft, rc)
            tmp = sb.tile([P, 512], f32, tag="ln_tmp")
            nc.vector.tensor_tensor(tmp, xt, psum_abc, AL.mult)
            nc.vector.tensor_tensor(
                dst[:, ft, rc * 512:(rc + 1) * 512], tmp, psum_bbc, AL.subtract
            )


def build_program(phases=("ln", "attn", "proj", "ffn")):
    nc = bass.Bass()
    xq_d = nc.dram_tensor("xq", [TQ, C], f32r, kind="ExternalInput")
    xkv_d = nc.dram_tensor("xkv", [T, C], f32r, kind="ExternalInput")
    mask_d = nc.dram_tensor("maskc", [4, 4, P, 256], f32, kind="ExternalInput")
    wq_d = nc.dram_tensor("wq", [8, P, 8, P], f32r, kind="ExternalInput")
    wk_d = nc.dram_tensor("wk", [8, P, 8, P], f32r, kind="ExternalInput")
    wv_d = nc.dram_tensor("wv", [8, P, 8, P], f32r, kind="ExternalInput")
    wp_d = nc.dram_tensor("wp", [C, C], f32r, kind="ExternalInput")
    w1_d = nc.dram_tensor("w1", [32, P, 8, P], f32r, kind="ExternalInput")
    w2_d = nc.dram_tensor("w2", [8, P, 32, P], f32r, kind="ExternalInput")
    bq_d = nc.dram_tensor("bq", [C], f32, kind="ExternalInput")
    bk_d = nc.dram_tensor("bk", [C], f32, kind="ExternalInput")
    bv_d = nc.dram_tensor("bv", [C], f32, kind="ExternalInput")
    bp_d = nc.dram_tensor("bp", [C], f32, kind="ExternalInput")
    b1_d = nc.dram_tensor("b1", [F], f32, kind="ExternalInput")
    b2_d = nc.dram_tensor("b2", [C], f32, kind="ExternalInput")
    ones_d = nc.dram_tensor("onesc", [P, 1], f32r, kind="ExternalInput")
    ident_d = nc.dram_tensor("identc", [P, P], f32r, kind="ExternalInput")
    y_d = nc.dram_tensor("y", [C, TQ], f32, kind="ExternalOutput")
    x2s_d = nc.dram_tensor("x2scratch", [C, TQ], f32r)
    xqs_d = nc.dram_tensor("xqscratch", [C, TQ], f32r)

    wp_r = wp_d.rearrange("(ko p) o -> p ko o", p=P)

    with tile.TileContext(nc) as tc:
        with tc.tile_pool(name="consts", bufs=1) as cpool, \
             tc.tile_pool(name="persist", bufs=1) as pers:
            ones = cpool.tile([P, 1], f32r)
            nc.sync.dma_start(ones, ones_d[:, :])
            ident = cpool.tile([P, P], f32r)
            nc.sync.dma_start(ident, ident_d[:, :])
            eps_sb = cpool.tile([1, 1], f32)
            nc.vector.memset(eps_sb, LN_EPS)
            bq_sb = cpool.tile([P, 8], f32)
            nc.sync.dma_start(bq_sb, bq_d.rearrange("(o p) -> p o", p=P))
            bk_sb = cpool.tile([P, 8], f32)
            nc.sync.dma_start(bk_sb, bk_d.rearrange("(o p) -> p o", p=P))
            bv_sb = cpool.tile([P, 8], f32)
            nc.sync.dma_start(bv_sb, bv_d.rearrange("(o p) -> p o", p=P))
            bp_sb = cpool.tile([P, 8], f32)
            nc.sync.dma_start(bp_sb, bp_d.rearrange("(o p) -> p o", p=P))
            b1_sb = cpool.tile([P, 32], f32)
            nc.sync.dma_start(b1_sb, b1_d.rearrange("(o p) -> p o", p=P))
            b2_sb = cpool.tile([P, 8], f32)
            nc.sync.dma_start(b2_sb, b2_d.rearrange("(o p) -> p o", p=P))

            OT = pers.tile([P, 8, TQ], f32r)       # attn out, transposed

            # ---------------- Phase 0 + A: LN1 and attention ----------------
            with tc.tile_pool(name="attn_sb", bufs=1) as apool:
                hkvT = apool.tile([P, 8, T], f32r)
                hqT = apool.tile([P, 8, TQ], f32r)
                with tc.tile_pool(name="ln_sb", bufs=4) as lnsb, \
                     tc.tile_pool(name="ln_small", bufs=4) as lnsmall, \
                     tc.tile_pool(name="ln_ps", bufs=4, space="PSUM") as lnps:
                    eps128 = lnsmall.tile([P, 1], f32, tag="eps128")
                    nc.vector.memset(eps128, LN_EPS)

                    def ln_row_tile(src_ap, dstT, rt, transpose_raw=None):
                        """Load one 128-row tile row-major, LN it, PE-transpose
                        into dstT[:, ft, rt*128...]. Optionally also transpose
                        the raw rows into transpose_raw slices."""
                        xrow = lnsb.tile([P, C], f32r, tag="xrow")
                        nc.sync.dma_start(xrow, src_ap)
                        stats = lnsmall.tile([P, 2, 6], f32, tag="stats")
                        for sg in range(2):
                            nc.vector.bn_stats(stats[:, sg, :], xrow[:, sg * 512:(sg + 1) * 512])
                        mv = lnsmall.tile([P, 2], f32, tag="mv")
                        nc.vector.bn_aggr(mv, stats)
                        rstd = lnsmall.tile([P, 1], f32, tag="rstd")
                        nc.scalar.activation(rstd, mv[:, 1:2], AF.Sqrt,
                                             bias=eps128, scale=1.0)
                        nc.vector.reciprocal(rstd, rstd)
                        hrow = lnsb.tile([P, C], f32r, tag="hrow")
                        nc.vector.tensor_scalar(hrow, xrow, mv[:, 0:1], rstd,
                                                op0=AL.subtract, op1=AL.mult)
                        for ft in range(8):
                            psum_t = lnps.tile([P, P], f32r, tag="tr")
                            nc.tensor.matmul(psum_t, hrow[:, ft * P:(ft + 1) * P],
                                             ident, is_transpose=True,
                                             start=True, stop=True)
                            nc.vector.tensor_copy(
                                dstT[:, ft, rt * P:(rt + 1) * P], psum_t)
                        if transpose_raw is not None:
                            for ft in range(8):
                                psum_t = lnps.tile([P, P], f32r, tag="tr")
                                nc.tensor.matmul(psum_t, xrow[:, ft * P:(ft + 1) * P],
                                                 ident, is_transpose=True,
                                                 start=True, stop=True)
                                xqt = lnsb.tile([P, P], f32r, tag="xqt")
                                nc.vector.tensor_copy(xqt, psum_t)
                                nc.sync.dma_start(
                                    xqs_d[ft * P:(ft + 1) * P,
                                          rt * P:(rt + 1) * P], xqt)

                    if "ln" in phases:
                        for rt in range(16):
                            ln_row_tile(xkv_d[rt * P:(rt + 1) * P, :], hkvT, rt)
                        for rt in range(8):
                            ln_row_tile(xq_d[rt * P:(rt + 1) * P, :], hqT, rt,
                                        transpose_raw=True)

                with tc.tile_pool(name="maskp", bufs=1) as maskp, \
                     tc.tile_pool(name="pair_w", bufs=2) as wpool, \
                     tc.tile_pool(name="pair_big", bufs=1) as gpool, \
                     tc.tile_pool(name="pt_sb", bufs=3) as ptpool, \
                     tc.tile_pool(name="o_sb", bufs=2) as opool, \
                     tc.tile_pool(name="ps_kqv", bufs=2, space="PSUM") as ps_kqv, \
                     tc.tile_pool(name="ps_score", bufs=3, space="PSUM") as ps_sc, \
                       tc.tile_pool(name="ps_bcast", bufs=1, space="PSUM") as ps_bc, \
                     tc.tile_pool(name="ps_o", bufs=2, space="PSUM") as ps_o:
                    mask_sb = maskp.tile([P, 4, 4, 256], f32)
                    for j in range(4):
                        for t in range(4):
                            nc.sync.dma_start(mask_sb[:, j, t, :], mask_d[j, t])
                    for g in (range(8) if "attn" in phases else []):
                        wk_t = wpool.tile([P, 8, P], f32r, tag="wk")
                        nc.sync.dma_start(wk_t, wk_d[g])
                        wq_t = wpool.tile([P, 8, P], f32r, tag="wqt")
                        nc.sync.dma_start(wq_t, wq_d[g])
                        wv_t = wpool.tile([P, 8, P], f32r, tag="wv")
                        nc.sync.dma_start(wv_t, wv_d[g])

                        KT = gpool.tile([P, T], f32r, tag="KT")
                        for rc in range(4):
                            psum = ps_kqv.tile([P, 512], f32, tag="kqv")
                            for k in range(8):
                                nc.tensor.matmul(
                                    psum, wk_t[:, k, :],
                                    hkvT[:, k, rc * 512:(rc + 1) * 512],
                                    start=(k == 0), stop=(k == 7))
                            nc.vector.tensor_scalar_add(
                                KT[:, rc * 512:(rc + 1) * 512], psum,
                                bk_sb[:, g:g + 1])
                        QT = gpool.tile([P, TQ], f32r, tag="QT")
                        for rc in range(2):
                            psum = ps_kqv.tile([P, 512], f32, tag="kqv")
                            for k in range(8):
                                nc.tensor.matmul(
                                    psum, wq_t[:, k, :],
                                    hqT[:, k, rc * 512:(rc + 1) * 512],
                                    start=(k == 0), stop=(k == 7))
                            nc.vector.tensor_scalar_add(
                                QT[:, rc * 512:(rc + 1) * 512], psum,
                                bq_sb[:, g:g + 1])
                        VT = gpool.tile([P, 4, 512], f32r, tag="VT")
                        for rc in range(4):
                            psum = ps_kqv.tile([P, 512], f32, tag="kqv")
                            for k in range(8):
                                nc.tensor.matmul(
                                    psum, wv_t[:, k, :],
                                    hkvT[:, k, rc * 512:(rc + 1) * 512],
                                    start=(k == 0), stop=(k == 7))
                            nc.vector.tensor_scalar_add(
                                VT[:, rc, :], psum, bv_sb[:, g:g + 1])
                        # V row-major (+ ones col per head) via PE transpose
                        vaug = gpool.tile([P, 16, 130], f32r, tag="vaug")
                        for kt in range(16):
                            psum_t = ps_kqv.tile([P, P], f32r, tag="kqv")
                            nc.tensor.matmul(
                                psum_t, VT[:, kt // 4, (kt % 4) * P:(kt % 4 + 1) * P],
                                ident, is_transpose=True, start=True, stop=True)
                            nc.vector.tensor_copy(vaug[:, kt, 0:64], psum_t[:, 0:64])
                            nc.vector.tensor_copy(vaug[:, kt, 65:129], psum_t[:, 64:128])
                            nc.vector.tensor_copy(vaug[:, kt, 64:65], ones[:, :])
                            nc.vector.tensor_copy(vaug[:, kt, 129:130], ones[:, :])

                        for hh in range(2):
                            base = 64 * hh
                            for j in range(4):
                                nkt = 4 * j + 4
                                psum_o = ps_o.tile([65, 256], f32, tag="po")
                                for kt in range(nkt):
                                    psum_s = ps_sc.tile([P, 256], f32, tag="sc")
                                    nc.tensor.matmul(
                                        psum_s,
                                        KT[base:base + 64, kt * P:(kt + 1) * P],
                                        QT[base:base + 64, j * 256:(j + 1) * 256],
                                        start=True, stop=True)
                                    pt = ptpool.tile([P, 256], f32r, tag="pt")
                                    if kt >= 4 * j:
                                        ssb = ptpool.tile([P, 256], f32, tag="ssb")
                                        nc.vector.scalar_tensor_tensor(
                                            ssb, psum_s, 1.0,
                                            mask_sb[:, j, kt - 4 * j, :],
                                            op0=AL.bypass, op1=AL.add)
                                        nc.scalar.activation(pt, ssb, AF.Exp)
                                    else:
                                        nc.scalar.activation(pt, psum_s, AF.Exp)
                                    nc.tensor.matmul(
                                        psum_o, vaug[:, kt, 65 * hh:65 * hh + 65],
                                        pt, start=(kt == 0), stop=(kt == nkt - 1))
                                o_sb = opool.tile([65, 256], f32, tag="osb")
                                nc.vector.tensor_copy(o_sb, psum_o)
                                rec = opool.tile([1, 256], f32r, tag="rec")
                                with nc.allow_low_precision(reason="f32r bits"):
                                    nc.vector.reciprocal(rec, o_sb[64:65, :])
                                psum_bc = ps_bc.tile([64, 256], f32, tag="bc")
                                nc.tensor.matmul(
                                    psum_bc, ones[0:1, 0:1].broadcast_to((1, 64)),
                                    rec, start=True, stop=True)
                                nc.vector.tensor_tensor(
                                    OT[base:base + 64, g, j * 256:(j + 1) * 256],
                                    o_sb[0:64, :], psum_bc, AL.mult)

            # ---------------- Phase B: proj + residual + LN2 ----------------
            with tc.tile_pool(name="late", bufs=1) as late:
              h2T = late.tile([P, 8, TQ], f32r)    # LN2 output (reuses attn space)
              with tc.tile_pool(name="proj_sb", bufs=1) as prpool, \
                 tc.tile_pool(name="proj_tmp", bufs=3) as prtmp, \
                 tc.tile_pool(name="ps_proj", bufs=2, space="PSUM") as ps_pr:
                wp_t = prpool.tile([P, 8, C], f32r)
                nc.sync.dma_start(wp_t, wp_r)
                xqT = prpool.tile([P, 8, TQ], f32r)
                for ft in range(8):
                    nc.sync.dma_start(xqT[:, ft, :], xqs_d[ft * P:(ft + 1) * P, :])
                for of in (range(8) if "proj" in phases else []):
                    for rc in range(2):
                        psum = ps_pr.tile([P, 512], f32, tag="pr")
                        for k in range(8):
                            nc.tensor.matmul(
                                psum, wp_t[:, k, of * P:(of + 1) * P],
                                OT[:, k, rc * 512:(rc + 1) * 512],
                                start=(k == 0), stop=(k == 7))
                        x2sb = prtmp.tile([P, 512], f32r, tag="x2sb")
                        nc.vector.scalar_tensor_tensor(
                            x2sb, psum, bp_sb[:, of:of + 1],
                            xqT[:, of, rc * 512:(rc + 1) * 512],
                            op0=AL.add, op1=AL.add)
                        nc.sync.dma_start(
                            x2s_d[of * P:(of + 1) * P,
                                  rc * 512:(rc + 1) * 512], x2sb)
                with tc.tile_pool(name="ln2_sb", bufs=2) as lnsb2, \
                     tc.tile_pool(name="ln2_small", bufs=1) as lnsmall2, \
                     tc.tile_pool(name="ln2_pss", bufs=1, space="PSUM") as lnpss2, \
                     tc.tile_pool(name="ln2_psb", bufs=2, space="PSUM") as lnpsb2:
                    def src_x2(ft, rc, _p=lnsb2):
                        xt = _p.tile([P, 512], f32r, tag="ln_x")
                        nc.sync.dma_start(
                            xt, x2s_d[ft * P:(ft + 1) * P,
                                      rc * 512:(rc + 1) * 512])
                        return xt
                    _emit_ln(nc, (lnsb2, lnsmall2, lnpss2, lnpsb2),
                             src_x2, h2T, 2, ones, eps_sb)

              # ---------------- Phase C: FFN + residual + store ---------------
              with tc.tile_pool(name="w2_sb", bufs=3) as w2pool, \
                 tc.tile_pool(name="ffn_sb", bufs=2) as fpool, \
                   tc.tile_pool(name="relu_sb", bufs=1) as rpool, \
                   tc.tile_pool(name="y_sb", bufs=3) as ypool, \
                   tc.tile_pool(name="ps_f1", bufs=2, space="PSUM") as ps_f1, \
                   tc.tile_pool(name="ps_f2", bufs=2, space="PSUM") as ps_f2:
                  for rc in (range(2) if "ffn" in phases else []):
                      relu1T = rpool.tile([P, 32, 512], f32r, tag="relu")
                      for fk in range(32):
                          w1_t = fpool.tile([P, 8, P], f32r, tag="w1")
                          nc.sync.dma_start(w1_t, w1_d[fk])
                          psum = ps_f1.tile([P, 512], f32, tag="f1")
                          for k in range(8):
                              nc.tensor.matmul(
                                  psum, w1_t[:, k, :],
                                  h2T[:, k, rc * 512:(rc + 1) * 512],
                                  start=(k == 0), stop=(k == 7))
                          nc.scalar.activation(relu1T[:, fk, :], psum, AF.Relu,
                                               bias=b1_sb[:, fk:fk + 1], scale=1.0)
                      for of in range(8):
                          w2_t = w2pool.tile([P, 32, P], f32r, tag="w2")
                          nc.sync.dma_start(w2_t, w2_d[of])
                          psum = ps_f2.tile([P, 512], f32, tag="f2")
                          for fk in range(32):
                              nc.tensor.matmul(psum, w2_t[:, fk, :], relu1T[:, fk, :],
                                               start=(fk == 0), stop=(fk == 31))
                          x2c = ypool.tile([P, 512], f32r, tag="x2c")
                          nc.sync.dma_start(
                              x2c, x2s_d[of * P:(of + 1) * P,
                                         rc * 512:(rc + 1) * 512])
                          y_sb = ypool.tile([P, 512], f32, tag="y")
                          nc.vector.scalar_tensor_tensor(
                              y_sb, psum, b2_sb[:, of:of + 1], x2c,
                              op0=AL.add, op1=AL.add)
                          nc.sync.dma_start(
                              y_d[of * P:(of + 1) * P,
                                  rc * 512:(rc + 1) * 512], y_sb)
    _split_sync_waits(nc)
    return nc


_PROGRAM = None


def _get_program():
    global _PROGRAM
    if _PROGRAM is None:
        _PROGRAM = build_program()
    return _PROGRAM


def _host_prep(x, Wk, Wq, Wv, Wproj, bproj, W1, b1, W2, b2, g1, beta1, g2, beta2):
    """Fold LN affine params into weights; build per-core shards."""
    x = np.asarray(x, np.float32)
    scale = 1.0 / np.sqrt(D)
    Wq_f = (g1[:, None] * np.asarray(Wq, np.float32)) * scale
    bq_f = (beta1 @ np.asarray(Wq, np.float32)) * scale
    Wk_f = g1[:, None] * np.asarray(Wk, np.float32)
    bk_f = beta1 @ np.asarray(Wk, np.float32)
    Wv_f = g1[:, None] * np.asarray(Wv, np.float32)
    bv_f = beta1 @ np.asarray(Wv, np.float32)
    W1_f = g2[:, None] * np.asarray(W1, np.float32)
    b1_f = np.asarray(b1, np.float32) + beta2 @ np.asarray(W1, np.float32)

    def tile_in_out(W, n_in, n_out):
        # [in, out] -> [n_out, 128, n_in, 128]: dram block per out-tile with
        # one contiguous 4KB+ run per partition
        return np.ascontiguousarray(
            np.asarray(W, np.float32).reshape(n_in, P, n_out, P).transpose(2, 1, 0, 3))

    common = {
        "wq": tile_in_out(Wq_f, 8, 8),
        "wk": tile_in_out(Wk_f, 8, 8),
        "wv": tile_in_out(Wv_f, 8, 8),
        "wp": np.ascontiguousarray(np.asarray(Wproj, np.float32)),
        "w1": tile_in_out(W1_f, 8, 32),
        "w2": tile_in_out(np.asarray(W2, np.float32), 32, 8),
        "bq": np.ascontiguousarray(bq_f, np.float32),
        "bk": np.ascontiguousarray(bk_f, np.float32),
        "bv": np.ascontiguousarray(bv_f, np.float32),
        "bp": np.ascontiguousarray(np.asarray(bproj, np.float32)),
        "b1": np.ascontiguousarray(b1_f, np.float32),
        "b2": np.ascontiguousarray(np.asarray(b2, np.float32)),
        "onesc": np.ones((P, 1), np.float32),
        "identc": np.eye(P, dtype=np.float32),
    }

    in_maps = []
    row_maps = []
    for core in range(8):
        b, s = core // 2, core % 2
        pairs = _chunk_pairs(s)
        rows = []
        for (a1, a2) in pairs:
            rows.extend(range(a1 * P, a1 * P + P))
            rows.extend(range(a2 * P, a2 * P + P))
        rows = np.array(rows)
        row_maps.append((b, rows))
        xq = np.ascontiguousarray(x[b][rows])
        # mask[j, t, k, q]: key pos 128*(4j+t)+k vs query pos rows[256j+q]
        mask = np.empty((4, 4, P, 256), np.float32)
        for j in range(4):
            qpos = rows[256 * j:256 * j + 256]
            for t in range(4):
                kpos = np.arange(P * (4 * j + t), P * (4 * j + t + 1))
                mask[j, t] = np.where(kpos[:, None] <= qpos[None, :], 0.0, NEG)
        in_maps.append({
            "xq": xq,
            "xkv": np.ascontiguousarray(x[b]),
            "maskc": mask,
            **common,
        })
    return in_maps, row_maps


def kernel(**inputs):
    nc = _get_program()
    in_maps, row_maps = _host_prep(**inputs)
    res = run_bass_kernel_spmd(nc, in_maps, core_ids=list(range(8)))
    out = np.empty((B, T, C), np.float32)
    for core in range(8):
        b, rows = row_maps[core]
        out[b][rows] = res.results[core]["y"].T
    return out

